# revision 11
# baseline (speedup 1.0000x reference)
"""AttentionPool2d Trainium2 kernel (8-core data parallel over batch).

Math (per batch item), exploiting that only query token 0 survives into the
output: tokens t = [mean(x); x_tokens] + pos_emb; v = t @ Wv.T + bv;
out[1:] = v[1:] @ Wc.T + bc; out[0] = softmax(q0·K/sqrt(hd)) V @ Wc.T + bc
with q0 = K = V = v (per head). So: compute vT = (Wv t.T + ...) in
[channel, token] layout, do the 1-query attention with mask matmuls,
substitute ctx into token-0 columns of vT, and run one out-projection
u.T @ Wc.T over all 50 tokens.

All matmuls fp16 (measured l2 rel err ~3e-4 per matmul on TRN2; fp32
accumulation in PSUM). pos_emb and bv are folded into a host-precomputed
vposT = (pos_emb @ Wv.T + bv).T added during the PSUM->SBUF copy.
"""

import numpy as np

import bass_rust
import concourse.bass as bass
import concourse.mybir as mybir
import concourse.tile as tile
from concourse.bass_utils import run_bass_kernel_spmd
from concourse.tile_scheduler import PROC_NAME_TO_IDX
from contextlib import ExitStack

# ---------------------------------------------------------------- constants
B, C, S = 256, 2048, 7
HW = S * S              # 49 spatial tokens
N = HW + 1              # 50 tokens incl. mean token
H, OUT = 32, 1024
HD = C // H             # 64
SCALE = HD ** -0.5
CORES = 8
IPC = B // CORES        # 32 items per core
GI = 8                  # items per group
G = IPC // GI           # 4 groups
TOK = IPC * N           # 1600 token columns per core
KC = C // 128           # 16 contraction chunks
JC = C // 128           # 16 output-channel chunks of v
NG = GI * N             # 400 moving columns per group
OC2 = OUT // 512        # 2 out-projection column chunks

F16 = mybir.dt.float16
F32 = mybir.dt.float32

N_PROCS = 27


# ------------------------------------------------------- tile/walrus patches
def _patched_drain_and_barrier(self, tick_clock, wait_clock):
    """Stock tail drain carries one wait per ticked proc; walrus here allows
    a single sync-wait per instruction. Funnel waits through SP nops."""
    nc = self.nc
    gc = tick_clock.global_clock
    ticks = [gc.peek_next(i) - 1 for i in range(N_PROCS)]
    live = [i for i in range(N_PROCS) if ticks[i] > 0]
    sp_clock = wait_clock.engine_clocks[PROC_NAME_TO_IDX["SP"]]
    for p in live:
        vc = bass_rust.VectorClock()
        vc.require_at_least(p, ticks[p])
        nop = nc.sync.nop(nofuse=True, hint="tail_wait_funnel")
        wait_clock.add_sem_waits(
            nop.ins, bass_rust.ScopedClock({None: vc}), cur_clock=sp_clock
        )
        sp_clock.require_at_least(None, p, ticks[p])
    drain_inst = nc.sync.drain()
    wait_clock.add_sem_waits(
        drain_inst.ins, bass_rust.ScopedClock({None: gc}), cur_clock=sp_clock
    )
    nc.all_engine_barrier()
    assert self.sems is not None
    popped = nc._tile_sem_poison_stack.pop()
    assert popped is self._sem_poison
    nc.clear_and_free_semaphores(list(self.sems.allocated().values()))
    nc.all_engine_barrier()


tile.TileContext._drain_and_barrier = _patched_drain_and_barrier


def fix_excess_waits(nc, max_waits=1):
    """Hoist excess per-instruction sync-waits onto injected same-engine
    NoOps placed immediately before the offender (engine streams run in
    basic-block order)."""
    for bb in nc.m.functions[0].blocks:
        insts = bb.instructions
        if not any(
            i.sync_info and i.sync_info.on_wait and len(i.sync_info.on_wait) > max_waits
            for i in insts
        ):
            continue
        out = []
        for inst in insts:
            si = inst.sync_info
            if si and si.on_wait and len(si.on_wait) > max_waits:
                waits = list(si.on_wait)
                extra, keep = waits[:-max_waits], waits[-max_waits:]
                for i in range(0, len(extra), max_waits):
                    chunk = extra[i : i + max_waits]
                    nop = mybir.InstNoOp(
                        name=nc.get_next_instruction_name(), ins=[], outs=[]
                    )
                    nop.engine = inst.engine
                    nop.sync_info = bass_rust.SyncInfo(on_wait=chunk, on_update=[])
                    nc.register_instruction(nop)
                    out.append(nop)
                si.on_wait = keep
            out.append(inst)
        bb.instructions = out


# ------------------------------------------------------------- kernel build
def build_kernel(reps=1):
    nc = bass.Bass("TRN2", target_bir_lowering=False, debug=False)

    x_d = nc.dram_tensor("x", [IPC, C, HW], F32, kind="ExternalInput")
    wv_d = nc.dram_tensor("wvT", [C, C], F16, kind="ExternalInput")
    wc_d = nc.dram_tensor("wcT", [C, OUT], F16, kind="ExternalInput")
    vpos_d = nc.dram_tensor("vposT", [128, KC * N], F32, kind="ExternalInput")
    maskT_d = nc.dram_tensor("maskT", [128, KC * H], F16, kind="ExternalInput")
    mask2_d = nc.dram_tensor("mask2", [H, KC * 128], F16, kind="ExternalInput")
    out_d = nc.dram_tensor("out", [IPC, N, OUT], F32, kind="ExternalOutput")
    out_flat = out_d.ap().rearrange("i n o -> (i n) o")

    with tile.TileContext(nc) as tc, ExitStack() as ctx:
        wv_pool = ctx.enter_context(tc.tile_pool(name="wv", bufs=1))
        wc_pool = ctx.enter_context(tc.tile_pool(name="wc", bufs=1))
        cpool = ctx.enter_context(tc.tile_pool(name="consts", bufs=1))
        xpool = ctx.enter_context(tc.tile_pool(name="xstage", bufs=3))
        tpool = ctx.enter_context(tc.tile_pool(name="tT", bufs=2))
        vpool = ctx.enter_context(tc.tile_pool(name="vT", bufs=1))
        apool = ctx.enter_context(tc.tile_pool(name="attn", bufs=2))
        opool = ctx.enter_context(tc.tile_pool(name="outsb", bufs=2))
        pv = ctx.enter_context(tc.tile_pool(name="pv", bufs=2, space="PSUM"))
        pS = ctx.enter_context(tc.tile_pool(name="pS", bufs=2, space="PSUM"))
        pA = ctx.enter_context(tc.tile_pool(name="pA", bufs=2, space="PSUM"))
        po = ctx.enter_context(tc.tile_pool(name="po", bufs=2, space="PSUM"))

        # ---- resident weights/constants
        wv_sb = []
        for kc in range(KC):
            w = wv_pool.tile([128, C], F16, name=f"wv{kc}", tag=f"wv{kc}")
            nc.sync.dma_start(w[:], wv_d.ap()[kc * 128 : (kc + 1) * 128, :])
            wv_sb.append(w)
        wc_sb = []
        for kc in range(KC):
            w = wc_pool.tile([128, OUT], F16, name=f"wc{kc}", tag=f"wc{kc}")
            nc.sync.dma_start(w[:], wc_d.ap()[kc * 128 : (kc + 1) * 128, :])
            wc_sb.append(w)
        vpos_sb = cpool.tile([128, KC * N], F32, name="vpos")
        nc.sync.dma_start(vpos_sb[:], vpos_d.ap())
        maskT_sb = cpool.tile([128, KC * H], F16, name="maskT")
        nc.sync.dma_start(maskT_sb[:], maskT_d.ap())
        mask2_sb = cpool.tile([H, KC * 128], F16, name="mask2")
        nc.sync.dma_start(mask2_sb[:], mask2_d.ap())

        # vT_all[jb]: [128, TOK] fp16, channel block jb x all token columns
        vT = []
        for jb in range(JC):
            v = vpool.tile([128, TOK], F16, name=f"vT{jb}", tag=f"vT{jb}")
            vT.append(v)

        def work():
            body(nc, tc, x_d, out_flat, wv_sb, wc_sb, vpos_sb, maskT_sb,
                 mask2_sb, vT, tpool, xpool, apool, opool, pv, pS, pA, po)

        if reps == 1:
            work()
        else:
            with tc.For_i(0, reps, 1):
                work()

    fix_excess_waits(nc)
    return nc


def body(nc, tc, x_d, out_flat, wv_sb, wc_sb, vpos_sb, maskT_sb, mask2_sb,
         vT, tpool, xpool, apool, opool, pv, pS, pA, po):
    if True:
        for g in range(G):
            g0 = g * NG  # first global token column of this group

            # ---- build tT for the group: [128, GI*(KC*N)] fp16,
            # per item blocks of KC*N=800 cols: block kc holds
            # [mean, x_0..x_48] for channels 128*kc..128*kc+127
            tT = tpool.tile([128, GI * KC * N], F16, name="tT", tag="tT")
            tT4 = tT[:].rearrange("p (i k n) -> p i k n", i=GI, k=KC)
            for it in range(GI):
                gi = g * GI + it
                xs = xpool.tile([128, KC * HW], F32, name="xs", tag="xs")
                xs3 = xs[:].rearrange("p (k n) -> p k n", k=KC)
                # x[gi] is [C, HW] row-major; channel chunk kc -> partition p
                nc.sync.dma_start(
                    xs[:],
                    x_d.ap()[gi].rearrange("(k p) n -> p k n", p=128),
                )
                # spatial tokens, cast to fp16
                nc.scalar.activation(
                    tT4[:, it, :, 1:N],
                    xs3,
                    mybir.ActivationFunctionType.Copy,
                )
                # mean token: reduce over the 49 spatial positions
                xsum = xpool.tile([128, KC], F32, name="xsum", tag="xsum")
                nc.vector.reduce_sum(xsum[:], xs3, axis=mybir.AxisListType.X)
                nc.scalar.activation(
                    tT4[:, it, :, 0],
                    xsum[:],
                    mybir.ActivationFunctionType.Copy,
                    scale=1.0 / HW,
                )

            # ---- v-projection: vT[jb][:, group cols] over 16 K-chunks
            vpos3 = vpos_sb[:].rearrange("p (k n) -> p k n", k=KC)
            for jb in range(JC):
                psum = pv.tile([128, NG], F32, name="pvt", tag="pvt")
                for kc in range(KC):
                    nc.tensor.matmul(
                        psum[:],
                        wv_sb[kc][:, jb * 128 : (jb + 1) * 128],
                        tT4[:, :, kc, :],
                        start=(kc == 0),
                        stop=(kc == KC - 1),
                    )
                # add vposT (same 50-col pattern for every item) + fp16 round
                nc.vector.tensor_add(
                    vT[jb][:, g0 : g0 + NG].rearrange("p (i n) -> p i n", i=GI),
                    psum[:].rearrange("p (i n) -> p i n", i=GI),
                    vpos3[:, jb : jb + 1, :].broadcast_to((128, GI, N)),
                )

            # ---- attention (batched over the group's 8 items)
            # P[jb][c, it*50+m] = vT[c, it*50+m] * vT[c, it*50+0]
            psum_S = pS.tile([H, NG], F32, name="psS", tag="psS")
            for jb in range(JC):
                vg3 = vT[jb][:, g0 : g0 + NG].rearrange("p (i n) -> p i n", i=GI)
                p = apool.tile([128, NG], F16, name="pprod", tag="pprod")
                nc.vector.tensor_mul(
                    p[:].rearrange("p (i n) -> p i n", i=GI),
                    vg3,
                    vg3[:, :, 0:1].broadcast_to((128, GI, N)),
                )
                nc.tensor.matmul(
                    psum_S[:],
                    maskT_sb[:, jb * H : (jb + 1) * H],
                    p[:],
                    start=(jb == 0),
                    stop=(jb == JC - 1),
                )
            # E = exp(S * scale), denominators per item block, A = E/D
            e_sb = apool.tile([H, NG], F32, name="esb", tag="esb")
            nc.scalar.activation(
                e_sb[:], psum_S[:], mybir.ActivationFunctionType.Exp, scale=SCALE
            )
            d_sb = apool.tile([H, GI], F32, name="dsb", tag="dsb")
            nc.vector.reduce_sum(
                d_sb[:],
                e_sb[:].rearrange("p (i n) -> p i n", i=GI),
                axis=mybir.AxisListType.X,
            )
            r_sb = apool.tile([H, GI], F32, name="rsb", tag="rsb")
            nc.vector.reciprocal(r_sb[:], d_sb[:])
            a_sb = apool.tile([H, NG], F16, name="asb", tag="asb")
            nc.vector.tensor_mul(
                a_sb[:].rearrange("p (i n) -> p i n", i=GI),
                e_sb[:].rearrange("p (i n) -> p i n", i=GI),
                r_sb[:].rearrange("p (i o) -> p i o", o=1).broadcast_to((H, GI, N)),
            )
            # ctx[c] = sum_m A[head(c), m] vT[c, m]; write into token-0 cols
            for jb in range(JC):
                psum_a = pA.tile([128, NG], F32, name="psA", tag="psA")
                nc.tensor.matmul(
                    psum_a[:],
                    mask2_sb[:, jb * 128 : (jb + 1) * 128],
                    a_sb[:],
                    start=True,
                    stop=True,
                )
                p2 = apool.tile([128, NG], F32, name="p2", tag="p2")
                nc.vector.tensor_mul(p2[:], psum_a[:], vT[jb][:, g0 : g0 + NG])
                ctx8 = apool.tile([128, GI], F32, name="ctx8", tag="ctx8")
                nc.vector.reduce_sum(
                    ctx8[:],
                    p2[:].rearrange("p (i n) -> p i n", i=GI),
                    axis=mybir.AxisListType.X,
                )
                nc.scalar.activation(
                    vT[jb][:, g0 : g0 + NG].rearrange("p (i n) -> p i n", i=GI)[
                        :, :, 0
                    ],
                    ctx8[:],
                    mybir.ActivationFunctionType.Copy,
                )

        # ---- out-projection: out[tok, :] = uT.T @ WcT over all 1600 tokens
        mtiles = [(m, min(128, TOK - m)) for m in range(0, TOK, 128)]
        for m0, mw in mtiles:
            osb = opool.tile([128, OUT], F32, name="osb", tag="osb")
            for oc in range(OC2):
                psum = po.tile([128, 512], F32, name="pso", tag="pso")
                for kc in range(KC):
                    nc.tensor.matmul(
                        psum[:mw, :],
                        vT[kc][:, m0 : m0 + mw],
                        wc_sb[kc][:, oc * 512 : (oc + 1) * 512],
                        start=(kc == 0),
                        stop=(kc == KC - 1),
                    )
                nc.vector.tensor_copy(
                    osb[:mw, oc * 512 : (oc + 1) * 512], psum[:mw, :]
                )
            nc.sync.dma_start(out_flat[m0 : m0 + mw, :], osb[:mw, :])


_NC_CACHE = None


def _get_nc():
    global _NC_CACHE
    if _NC_CACHE is None:
        _NC_CACHE = build_kernel()
    return _NC_CACHE


# ---------------------------------------------------------------- host side
def make_in_maps(inputs):
    x = np.asarray(inputs["x"], np.float32)
    pos_emb = np.asarray(inputs["pos_emb"], np.float32)
    Wv = np.asarray(inputs["Wv"], np.float32)
    bv = np.asarray(inputs["bv"], np.float32)
    Wc = np.asarray(inputs["Wc"], np.float32)
    bc = np.asarray(inputs["bc"], np.float32)
    num_heads = int(np.asarray(inputs["num_heads"]))
    assert num_heads == H and x.shape == (B, C, S, S)

    wvT = np.ascontiguousarray(Wv.T).astype(np.float16)
    wcT = np.ascontiguousarray(Wc.T).astype(np.float16)

    # vposT[128, kc*50 + n] = (pos_emb @ Wv.T + bv).T chunk-tiled
    vpos = (pos_emb @ Wv.T + bv).astype(np.float32)  # [N, C]
    vposT = np.empty((128, KC * N), np.float32)
    for kc in range(KC):
        vposT[:, kc * N : (kc + 1) * N] = vpos[:, kc * 128 : (kc + 1) * 128].T

    # maskT[p, kc*32 + h] = 1 if channel kc*128+p belongs to head h
    ch = np.arange(C)
    head_of = ch // HD
    maskT = np.zeros((128, KC * H), np.float16)
    mask2 = np.zeros((H, KC * 128), np.float16)
    for kc in range(KC):
        for p in range(128):
            h = head_of[kc * 128 + p]
            maskT[p, kc * H + h] = 1.0
            mask2[h, kc * 128 + p] = 1.0

    xr = x.reshape(B, C, HW)
    in_maps = []
    for core in range(CORES):
        in_maps.append(
            {
                "x": np.ascontiguousarray(xr[core * IPC : (core + 1) * IPC]),
                "wvT": wvT,
                "wcT": wcT,
                "vposT": vposT,
                "maskT": maskT,
                "mask2": mask2,
            }
        )

    return in_maps


def kernel(**inputs):
    in_maps = make_in_maps(inputs)
    nc = _get_nc()
    res = run_bass_kernel_spmd(nc, in_maps, list(range(CORES)))
    out = np.concatenate([res.results[i]["out"] for i in range(CORES)], axis=0)
    out = np.ascontiguousarray(out, dtype=np.float32)
    bc = np.asarray(inputs["bc"], np.float32)
    if bc.any():
        out = out + bc[None, None, :]
    return out


# revision 12
# speedup vs baseline: 1.8434x; 1.8434x over previous
"""AttentionPool2d Trainium2 kernel (8-core data parallel over batch).

Math (per batch item), exploiting that only query token 0 survives into the
output: tokens t = [mean(x); x_tokens] + pos_emb; v = t @ Wv.T + bv;
out[1:] = v[1:] @ Wc.T + bc; out[0] = softmax(q0·K/sqrt(hd)) V @ Wc.T + bc
with q0 = K = V = v (per head). So: compute vT = (Wv t.T + ...) in
[channel, token] layout, do the 1-query attention with mask matmuls,
substitute ctx into token-0 columns of vT, and run one out-projection
u.T @ Wc.T over all 50 tokens.

All matmuls fp16 (measured l2 rel err ~3e-4 per matmul on TRN2; fp32
accumulation in PSUM). pos_emb and bv are folded into a host-precomputed
vposT = (pos_emb @ Wv.T + bv).T added during the PSUM->SBUF copy.
"""

import numpy as np

import bass_rust
import concourse.bass as bass
import concourse.mybir as mybir
import concourse.tile as tile
from concourse.bass_utils import run_bass_kernel_spmd
from concourse.tile_scheduler import PROC_NAME_TO_IDX
from contextlib import ExitStack

# ---------------------------------------------------------------- constants
B, C, S = 256, 2048, 7
HW = S * S              # 49 spatial tokens
N = HW + 1              # 50 tokens incl. mean token
H, OUT = 32, 1024
HD = C // H             # 64
SCALE = HD ** -0.5
CORES = 8
IPC = B // CORES        # 32 items per core
GI = 8                  # items per group
G = IPC // GI           # 4 groups
TOK = IPC * N           # 1600 token columns per core
KC = C // 128           # 16 contraction chunks
JC = C // 128           # 16 output-channel chunks of v
NG = GI * N             # 400 moving columns per group
OC2 = OUT // 512        # 2 out-projection column chunks

F16 = mybir.dt.float16
F32 = mybir.dt.float32

N_PROCS = 27


# ------------------------------------------------------- tile/walrus patches
def _patched_drain_and_barrier(self, tick_clock, wait_clock):
    """Stock tail drain carries one wait per ticked proc; walrus here allows
    a single sync-wait per instruction. Funnel waits through SP nops."""
    nc = self.nc
    gc = tick_clock.global_clock
    ticks = [gc.peek_next(i) - 1 for i in range(N_PROCS)]
    live = [i for i in range(N_PROCS) if ticks[i] > 0]
    sp_clock = wait_clock.engine_clocks[PROC_NAME_TO_IDX["SP"]]
    for p in live:
        vc = bass_rust.VectorClock()
        vc.require_at_least(p, ticks[p])
        nop = nc.sync.nop(nofuse=True, hint="tail_wait_funnel")
        wait_clock.add_sem_waits(
            nop.ins, bass_rust.ScopedClock({None: vc}), cur_clock=sp_clock
        )
        sp_clock.require_at_least(None, p, ticks[p])
    drain_inst = nc.sync.drain()
    wait_clock.add_sem_waits(
        drain_inst.ins, bass_rust.ScopedClock({None: gc}), cur_clock=sp_clock
    )
    nc.all_engine_barrier()
    assert self.sems is not None
    popped = nc._tile_sem_poison_stack.pop()
    assert popped is self._sem_poison
    nc.clear_and_free_semaphores(list(self.sems.allocated().values()))
    nc.all_engine_barrier()


tile.TileContext._drain_and_barrier = _patched_drain_and_barrier


def fix_excess_waits(nc, max_waits=1):
    """Hoist excess per-instruction sync-waits onto injected same-engine
    NoOps placed immediately before the offender (engine streams run in
    basic-block order)."""
    for bb in nc.m.functions[0].blocks:
        insts = bb.instructions
        if not any(
            i.sync_info and i.sync_info.on_wait and len(i.sync_info.on_wait) > max_waits
            for i in insts
        ):
            continue
        out = []
        for inst in insts:
            si = inst.sync_info
            if si and si.on_wait and len(si.on_wait) > max_waits:
                waits = list(si.on_wait)
                extra, keep = waits[:-max_waits], waits[-max_waits:]
                for i in range(0, len(extra), max_waits):
                    chunk = extra[i : i + max_waits]
                    nop = mybir.InstNoOp(
                        name=nc.get_next_instruction_name(), ins=[], outs=[]
                    )
                    nop.engine = inst.engine
                    nop.sync_info = bass_rust.SyncInfo(on_wait=chunk, on_update=[])
                    nc.register_instruction(nop)
                    out.append(nop)
                si.on_wait = keep
            out.append(inst)
        bb.instructions = out


# ------------------------------------------------------------- kernel build
def build_kernel(reps=1):
    nc = bass.Bass("TRN2", target_bir_lowering=False, debug=False)

    x_d = nc.dram_tensor("x", [IPC, C, HW], F32, kind="ExternalInput")
    wv_d = nc.dram_tensor("wvT", [C, C], F16, kind="ExternalInput")
    wc_d = nc.dram_tensor("wcT", [C, OUT], F16, kind="ExternalInput")
    vpos_d = nc.dram_tensor("vposT", [128, KC * N], F32, kind="ExternalInput")
    maskT_d = nc.dram_tensor("maskT", [128, KC * H], F16, kind="ExternalInput")
    mask2_d = nc.dram_tensor("mask2", [H, KC * 128], F16, kind="ExternalInput")
    out_d = nc.dram_tensor("out", [IPC, N, OUT], F32, kind="ExternalOutput")
    out_flat = out_d.ap().rearrange("i n o -> (i n) o")

    with tile.TileContext(nc) as tc, ExitStack() as ctx:
        wv_pool = ctx.enter_context(tc.tile_pool(name="wv", bufs=1))
        wc_pool = ctx.enter_context(tc.tile_pool(name="wc", bufs=1))
        cpool = ctx.enter_context(tc.tile_pool(name="consts", bufs=1))
        xpool = ctx.enter_context(tc.tile_pool(name="xstage", bufs=3))
        tpool = ctx.enter_context(tc.tile_pool(name="tT", bufs=2))
        vpool = ctx.enter_context(tc.tile_pool(name="vT", bufs=1))
        apool = ctx.enter_context(tc.tile_pool(name="attn", bufs=2))
        opool = ctx.enter_context(tc.tile_pool(name="outsb", bufs=2))
        pv = ctx.enter_context(tc.tile_pool(name="pv", bufs=2, space="PSUM"))
        pS = ctx.enter_context(tc.tile_pool(name="pS", bufs=2, space="PSUM"))
        pA = ctx.enter_context(tc.tile_pool(name="pA", bufs=2, space="PSUM"))
        po = ctx.enter_context(tc.tile_pool(name="po", bufs=2, space="PSUM"))

        # ---- resident weights/constants
        wv_sb = []
        for kc in range(KC):
            w = wv_pool.tile([128, C], F16, name=f"wv{kc}", tag=f"wv{kc}")
            nc.sync.dma_start(w[:], wv_d.ap()[kc * 128 : (kc + 1) * 128, :])
            wv_sb.append(w)
        wc_sb = []
        for kc in range(KC):
            w = wc_pool.tile([128, OUT], F16, name=f"wc{kc}", tag=f"wc{kc}")
            nc.sync.dma_start(w[:], wc_d.ap()[kc * 128 : (kc + 1) * 128, :])
            wc_sb.append(w)
        vpos_sb = cpool.tile([128, KC * N], F32, name="vpos")
        nc.sync.dma_start(vpos_sb[:], vpos_d.ap())
        maskT_sb = cpool.tile([128, KC * H], F16, name="maskT")
        nc.sync.dma_start(maskT_sb[:], maskT_d.ap())
        mask2_sb = cpool.tile([H, KC * 128], F16, name="mask2")
        nc.sync.dma_start(mask2_sb[:], mask2_d.ap())

        # vT_all[jb]: [128, TOK] fp16, channel block jb x all token columns
        vT = []
        for jb in range(JC):
            v = vpool.tile([128, TOK], F16, name=f"vT{jb}", tag=f"vT{jb}")
            vT.append(v)

        def work():
            body(nc, tc, x_d, out_flat, wv_sb, wc_sb, vpos_sb, maskT_sb,
                 mask2_sb, vT, tpool, xpool, apool, opool, pv, pS, pA, po)

        if reps == 1:
            work()
        else:
            with tc.For_i(0, reps, 1):
                work()

    fix_excess_waits(nc)
    return nc


def body(nc, tc, x_d, out_flat, wv_sb, wc_sb, vpos_sb, maskT_sb, mask2_sb,
         vT, tpool, xpool, apool, opool, pv, pS, pA, po):
    vpos3 = vpos_sb[:].rearrange("p (k n) -> p k n", k=KC)

    def build_tT(g):
        # tT layout: [128, KC*(GI*N)] fp16 -- kc-major blocks of 400 cols so
        # the matmul moving operand is contiguous: col = kc*400 + it*50 + n
        tT = tpool.tile([128, KC * GI * N], F16, name="tT", tag="tT")
        tT4 = tT[:].rearrange("p (k i n) -> p k i n", k=KC, i=GI)
        for it in range(GI):
            gi = g * GI + it
            xs = xpool.tile([128, KC * HW], F32, name="xs", tag="xs")
            xs3 = xs[:].rearrange("p (k n) -> p k n", k=KC)
            # x[gi] is [C, HW] row-major; channel chunk kc -> partition p
            nc.sync.dma_start(
                xs[:],
                x_d.ap()[gi].rearrange("(k p) n -> p k n", p=128),
            )
            # spatial tokens, cast to fp16
            nc.scalar.activation(
                tT4[:, :, it, 1:N],
                xs3,
                mybir.ActivationFunctionType.Copy,
            )
            # mean token: reduce over the 49 spatial positions
            xsum = xpool.tile([128, KC], F32, name="xsum", tag="xsum")
            nc.vector.reduce_sum(xsum[:], xs3, axis=mybir.AxisListType.X)
            nc.scalar.activation(
                tT4[:, :, it, 0],
                xsum[:],
                mybir.ActivationFunctionType.Copy,
                scale=1.0 / HW,
            )
        return tT

    def vproj(g, tT):
        g0 = g * NG
        for jb in range(JC):
            psum = pv.tile([128, NG], F32, name="pvt", tag="pvt")
            for kc in range(KC):
                nc.tensor.matmul(
                    psum[:],
                    wv_sb[kc][:, jb * 128 : (jb + 1) * 128],
                    tT[:, kc * NG : (kc + 1) * NG],
                    start=(kc == 0),
                    stop=(kc == KC - 1),
                )
            # add vposT (same 50-col pattern for every item) + fp16 round
            nc.vector.tensor_add(
                vT[jb][:, g0 : g0 + NG].rearrange("p (i n) -> p i n", i=GI),
                psum[:].rearrange("p (i n) -> p i n", i=GI),
                vpos3[:, jb : jb + 1, :].broadcast_to((128, GI, N)),
            )

    def attention(g):
        g0 = g * NG
        # P[jb][c, it*50+m] = vT[c, it*50+m] * vT[c, it*50+0]
        psum_S = pS.tile([H, NG], F32, name="psS", tag="psS")
        for jb in range(JC):
            vg3 = vT[jb][:, g0 : g0 + NG].rearrange("p (i n) -> p i n", i=GI)
            p = apool.tile([128, NG], F16, name="pprod", tag="pprod")
            nc.vector.tensor_mul(
                p[:].rearrange("p (i n) -> p i n", i=GI),
                vg3,
                vg3[:, :, 0:1].broadcast_to((128, GI, N)),
            )
            nc.tensor.matmul(
                psum_S[:],
                maskT_sb[:, jb * H : (jb + 1) * H],
                p[:],
                start=(jb == 0),
                stop=(jb == JC - 1),
            )
        # E = exp(S * scale), denominators per item block, A = E/D
        e_sb = apool.tile([H, NG], F32, name="esb", tag="esb")
        nc.scalar.activation(
            e_sb[:], psum_S[:], mybir.ActivationFunctionType.Exp, scale=SCALE
        )
        d_sb = apool.tile([H, GI], F32, name="dsb", tag="dsb")
        nc.vector.reduce_sum(
            d_sb[:],
            e_sb[:].rearrange("p (i n) -> p i n", i=GI),
            axis=mybir.AxisListType.X,
        )
        r_sb = apool.tile([H, GI], F32, name="rsb", tag="rsb")
        nc.vector.reciprocal(r_sb[:], d_sb[:])
        a_sb = apool.tile([H, NG], F16, name="asb", tag="asb")
        nc.vector.tensor_mul(
            a_sb[:].rearrange("p (i n) -> p i n", i=GI),
            e_sb[:].rearrange("p (i n) -> p i n", i=GI),
            r_sb[:].rearrange("p (i o) -> p i o", o=1).broadcast_to((H, GI, N)),
        )
        # ctx[c] = sum_m A[head(c), m] vT[c, m]; write into token-0 cols
        for jb in range(JC):
            psum_a = pA.tile([128, NG], F32, name="psA", tag="psA")
            nc.tensor.matmul(
                psum_a[:],
                mask2_sb[:, jb * 128 : (jb + 1) * 128],
                a_sb[:],
                start=True,
                stop=True,
            )
            p2 = apool.tile([128, NG], F32, name="p2", tag="p2")
            nc.vector.tensor_mul(p2[:], psum_a[:], vT[jb][:, g0 : g0 + NG])
            ctx8 = apool.tile([128, GI], F32, name="ctx8", tag="ctx8")
            nc.vector.reduce_sum(
                ctx8[:],
                p2[:].rearrange("p (i n) -> p i n", i=GI),
                axis=mybir.AxisListType.X,
            )
            nc.scalar.activation(
                vT[jb][:, g0 : g0 + NG].rearrange("p (i n) -> p i n", i=GI)[
                    :, :, 0
                ],
                ctx8[:],
                mybir.ActivationFunctionType.Copy,
            )

    def outproj(mtiles):
        # out[tok, :] = uT.T @ WcT; token-stationary, 128 tokens per tile
        for m0, mw in mtiles:
            osb = opool.tile([128, OUT], F32, name="osb", tag="osb")
            for oc in range(OC2):
                psum = po.tile([128, 512], F32, name="pso", tag="pso")
                for kc in range(KC):
                    nc.tensor.matmul(
                        psum[:mw, :],
                        vT[kc][:, m0 : m0 + mw],
                        wc_sb[kc][:, oc * 512 : (oc + 1) * 512],
                        start=(kc == 0),
                        stop=(kc == KC - 1),
                    )
                nc.vector.tensor_copy(
                    osb[:mw, oc * 512 : (oc + 1) * 512], psum[:mw, :]
                )
            nc.sync.dma_start(out_flat[m0 : m0 + mw, :], osb[:mw, :])

    # Software-pipelined schedule: attention(g) PE work hides under
    # vproj(g+1); out-projection for tokens of groups 0..2 starts before
    # the last group attention completes.
    mtiles = [(m, min(128, TOK - m)) for m in range(0, TOK, 128)]
    early = [mt for mt in mtiles if mt[0] + mt[1] <= 3 * NG]
    late = [mt for mt in mtiles if mt[0] + mt[1] > 3 * NG]

    tT0 = build_tT(0)
    vproj(0, tT0)
    tT1 = build_tT(1)
    vproj(1, tT1)
    attention(0)
    tT2 = build_tT(2)
    vproj(2, tT2)
    attention(1)
    tT3 = build_tT(3)
    vproj(3, tT3)
    attention(2)
    outproj(early)
    attention(3)
    outproj(late)


_NC_CACHE = None


def _get_nc():
    global _NC_CACHE
    if _NC_CACHE is None:
        _NC_CACHE = build_kernel()
    return _NC_CACHE


# ---------------------------------------------------------------- host side
def make_in_maps(inputs):
    x = np.asarray(inputs["x"], np.float32)
    pos_emb = np.asarray(inputs["pos_emb"], np.float32)
    Wv = np.asarray(inputs["Wv"], np.float32)
    bv = np.asarray(inputs["bv"], np.float32)
    Wc = np.asarray(inputs["Wc"], np.float32)
    bc = np.asarray(inputs["bc"], np.float32)
    num_heads = int(np.asarray(inputs["num_heads"]))
    assert num_heads == H and x.shape == (B, C, S, S)

    wvT = np.ascontiguousarray(Wv.T).astype(np.float16)
    wcT = np.ascontiguousarray(Wc.T).astype(np.float16)

    # vposT[128, kc*50 + n] = (pos_emb @ Wv.T + bv).T chunk-tiled
    vpos = (pos_emb @ Wv.T + bv).astype(np.float32)  # [N, C]
    vposT = np.empty((128, KC * N), np.float32)
    for kc in range(KC):
        vposT[:, kc * N : (kc + 1) * N] = vpos[:, kc * 128 : (kc + 1) * 128].T

    # maskT[p, kc*32 + h] = 1 if channel kc*128+p belongs to head h
    ch = np.arange(C)
    head_of = ch // HD
    maskT = np.zeros((128, KC * H), np.float16)
    mask2 = np.zeros((H, KC * 128), np.float16)
    for kc in range(KC):
        for p in range(128):
            h = head_of[kc * 128 + p]
            maskT[p, kc * H + h] = 1.0
            mask2[h, kc * 128 + p] = 1.0

    xr = x.reshape(B, C, HW)
    in_maps = []
    for core in range(CORES):
        in_maps.append(
            {
                "x": np.ascontiguousarray(xr[core * IPC : (core + 1) * IPC]),
                "wvT": wvT,
                "wcT": wcT,
                "vposT": vposT,
                "maskT": maskT,
                "mask2": mask2,
            }
        )

    return in_maps


def kernel(**inputs):
    in_maps = make_in_maps(inputs)
    nc = _get_nc()
    res = run_bass_kernel_spmd(nc, in_maps, list(range(CORES)))
    out = np.concatenate([res.results[i]["out"] for i in range(CORES)], axis=0)
    out = np.ascontiguousarray(out, dtype=np.float32)
    bc = np.asarray(inputs["bc"], np.float32)
    if bc.any():
        out = out + bc[None, None, :]
    return out


# revision 20
# speedup vs baseline: 1.9722x; 1.0699x over previous
"""AttentionPool2d Trainium2 kernel (8-core data parallel over batch).

Math (per batch item), exploiting that only query token 0 survives into the
output: tokens t = [mean(x); x_tokens] + pos_emb; v = t @ Wv.T + bv;
out[1:] = v[1:] @ Wc.T + bc; out[0] = softmax(q0·K/sqrt(hd)) V @ Wc.T + bc
with q0 = K = V = v (per head). So: compute vT = (Wv t.T + ...) in
[channel, token] layout, do the 1-query attention with mask matmuls,
substitute ctx into token-0 columns of vT, and run one out-projection
u.T @ Wc.T over all 50 tokens.

All matmuls fp16 (measured l2 rel err ~3e-4 per matmul on TRN2; fp32
accumulation in PSUM). pos_emb and bv are folded into a host-precomputed
vposT = (pos_emb @ Wv.T + bv).T added during the PSUM->SBUF copy.
"""

import numpy as np

import bass_rust
import concourse.bass as bass
import concourse.mybir as mybir
import concourse.tile as tile
from concourse.bass_utils import run_bass_kernel_spmd
from concourse.tile_scheduler import PROC_NAME_TO_IDX
from contextlib import ExitStack

# ---------------------------------------------------------------- constants
B, C, S = 256, 2048, 7
HW = S * S              # 49 spatial tokens
N = HW + 1              # 50 tokens incl. mean token
H, OUT = 32, 1024        # default num_heads; build is parameterized
HD = C // H
SCALE = HD ** -0.5
CORES = 8
IPC = B // CORES        # 32 items per core
GI = 8                  # items per group
G = IPC // GI           # 4 groups
TOK = IPC * N           # 1600 token columns per core
KC = C // 128           # 16 contraction chunks
JC = C // 128           # 16 output-channel chunks of v
NG = GI * N             # 400 moving columns per group
OC2 = OUT // 512        # 2 out-projection column chunks

F16 = mybir.dt.float16
F32 = mybir.dt.float32

N_PROCS = 27


# ------------------------------------------------------- tile/walrus patches
def _patched_drain_and_barrier(self, tick_clock, wait_clock):
    """Stock tail drain carries one wait per ticked proc; walrus here allows
    a single sync-wait per instruction. Funnel waits through SP nops."""
    nc = self.nc
    gc = tick_clock.global_clock
    ticks = [gc.peek_next(i) - 1 for i in range(N_PROCS)]
    live = [i for i in range(N_PROCS) if ticks[i] > 0]
    sp_clock = wait_clock.engine_clocks[PROC_NAME_TO_IDX["SP"]]
    for p in live:
        vc = bass_rust.VectorClock()
        vc.require_at_least(p, ticks[p])
        nop = nc.sync.nop(nofuse=True, hint="tail_wait_funnel")
        wait_clock.add_sem_waits(
            nop.ins, bass_rust.ScopedClock({None: vc}), cur_clock=sp_clock
        )
        sp_clock.require_at_least(None, p, ticks[p])
    drain_inst = nc.sync.drain()
    wait_clock.add_sem_waits(
        drain_inst.ins, bass_rust.ScopedClock({None: gc}), cur_clock=sp_clock
    )
    nc.all_engine_barrier()
    assert self.sems is not None
    popped = nc._tile_sem_poison_stack.pop()
    assert popped is self._sem_poison
    nc.clear_and_free_semaphores(list(self.sems.allocated().values()))
    nc.all_engine_barrier()


tile.TileContext._drain_and_barrier = _patched_drain_and_barrier


def fix_excess_waits(nc, max_waits=1):
    """Hoist excess per-instruction sync-waits onto injected same-engine
    NoOps placed immediately before the offender (engine streams run in
    basic-block order)."""
    for bb in nc.m.functions[0].blocks:
        insts = bb.instructions
        if not any(
            i.sync_info and i.sync_info.on_wait and len(i.sync_info.on_wait) > max_waits
            for i in insts
        ):
            continue
        out = []
        for inst in insts:
            si = inst.sync_info
            if si and si.on_wait and len(si.on_wait) > max_waits:
                waits = list(si.on_wait)
                extra, keep = waits[:-max_waits], waits[-max_waits:]
                for i in range(0, len(extra), max_waits):
                    chunk = extra[i : i + max_waits]
                    nop = mybir.InstNoOp(
                        name=nc.get_next_instruction_name(), ins=[], outs=[]
                    )
                    nop.engine = inst.engine
                    nop.sync_info = bass_rust.SyncInfo(on_wait=chunk, on_update=[])
                    nc.register_instruction(nop)
                    out.append(nop)
                si.on_wait = keep
            out.append(inst)
        bb.instructions = out


# ------------------------------------------------------------- kernel build
def build_kernel(reps=1, variant="full", heads=H):
    nc = bass.Bass("TRN2", target_bir_lowering=False, debug=False)

    x_d = nc.dram_tensor("x", [IPC, C, HW], F16, kind="ExternalInput")
    wv_d = nc.dram_tensor("wvT", [C, C], F16, kind="ExternalInput")
    wc_d = nc.dram_tensor("wcT", [C, OUT], F16, kind="ExternalInput")
    vpos_d = nc.dram_tensor("vposT", [128, KC * N], F32, kind="ExternalInput")
    maskT_d = nc.dram_tensor("maskT", [128, KC * heads], F16, kind="ExternalInput")
    mask2_d = nc.dram_tensor("mask2", [heads, KC * 128], F16, kind="ExternalInput")
    out_d = nc.dram_tensor("out", [IPC, N, OUT], F32, kind="ExternalOutput")
    out_flat = out_d.ap().rearrange("i n o -> (i n) o")

    with tile.TileContext(nc) as tc, ExitStack() as ctx:
        wv_pool = ctx.enter_context(tc.tile_pool(name="wv", bufs=1))
        wc_pool = ctx.enter_context(tc.tile_pool(name="wc", bufs=1))
        cpool = ctx.enter_context(tc.tile_pool(name="consts", bufs=1))
        xpool = ctx.enter_context(tc.tile_pool(name="xstage", bufs=3))
        tpool = ctx.enter_context(tc.tile_pool(name="tT", bufs=2))
        vpool = ctx.enter_context(tc.tile_pool(name="vT", bufs=1))
        apool = ctx.enter_context(tc.tile_pool(name="attn", bufs=2))
        opool = ctx.enter_context(tc.tile_pool(name="outsb", bufs=2))
        pv = ctx.enter_context(tc.tile_pool(name="pv", bufs=2, space="PSUM"))
        pS = ctx.enter_context(tc.tile_pool(name="pS", bufs=2, space="PSUM"))
        pA = ctx.enter_context(tc.tile_pool(name="pA", bufs=2, space="PSUM"))
        po = ctx.enter_context(tc.tile_pool(name="po", bufs=2, space="PSUM"))

        # ---- resident weights/constants
        wv_sb = []
        for kc in range(KC):
            w = wv_pool.tile([128, C], F16, name=f"wv{kc}", tag=f"wv{kc}")
            nc.sync.dma_start(w[:], wv_d.ap()[kc * 128 : (kc + 1) * 128, :])
            wv_sb.append(w)
        wc_sb = []
        for kc in range(KC):
            w = wc_pool.tile([128, OUT], F16, name=f"wc{kc}", tag=f"wc{kc}")
            nc.sync.dma_start(w[:], wc_d.ap()[kc * 128 : (kc + 1) * 128, :])
            wc_sb.append(w)
        vpos_sb = cpool.tile([128, KC * N], F32, name="vpos")
        nc.sync.dma_start(vpos_sb[:], vpos_d.ap())
        maskT_sb = cpool.tile([128, KC * heads], F16, name="maskT")
        nc.sync.dma_start(maskT_sb[:], maskT_d.ap())
        mask2_sb = cpool.tile([heads, KC * 128], F16, name="mask2")
        nc.sync.dma_start(mask2_sb[:], mask2_d.ap())

        # vT_all[jb]: [128, TOK] fp16, channel block jb x all token columns
        vT = []
        for jb in range(JC):
            v = vpool.tile([128, TOK], F16, name=f"vT{jb}", tag=f"vT{jb}")
            vT.append(v)

        def work():
            body(nc, tc, x_d, out_flat, wv_sb, wc_sb, vpos_sb, maskT_sb,
                 mask2_sb, vT, tpool, xpool, apool, opool, pv, pS, pA, po,
                 variant, heads)

        if reps == 1:
            work()
        else:
            with tc.For_i(0, reps, 1):
                work()

    fix_excess_waits(nc)
    return nc


def body(nc, tc, x_d, out_flat, wv_sb, wc_sb, vpos_sb, maskT_sb, mask2_sb,
         vT, tpool, xpool, apool, opool, pv, pS, pA, po, variant="full",
         heads=H):
    scale = (C // heads) ** -0.5
    vpos3 = vpos_sb[:].rearrange("p (k n) -> p k n", k=KC)

    def build_tT(g):
        # tT layout: [128, KC*(GI*N)] fp16 -- kc-major blocks of 400 cols so
        # the matmul moving operand is contiguous: col = kc*400 + it*50 + n
        tT = tpool.tile([128, KC * GI * N], F16, name="tT", tag="tT")
        tT4 = tT[:].rearrange("p (k i n) -> p k i n", k=KC, i=GI)
        for it in range(GI):
            gi = g * GI + it
            xs = xpool.tile([128, KC * HW], F16, name="xs", tag="xs")
            xs3 = xs[:].rearrange("p (k n) -> p k n", k=KC)
            # x[gi] is [C, HW] row-major; channel chunk kc -> partition p
            nc.sync.dma_start(
                xs[:],
                x_d.ap()[gi].rearrange("(k p) n -> p k n", p=128),
            )
            # spatial tokens, cast to fp16
            nc.scalar.activation(
                tT4[:, :, it, 1:N],
                xs3,
                mybir.ActivationFunctionType.Copy,
            )
            # mean token: reduce over the 49 spatial positions
            xsum = xpool.tile([128, KC], F32, name="xsum", tag="xsum")
            nc.vector.reduce_sum(xsum[:], xs3, axis=mybir.AxisListType.X)
            nc.scalar.activation(
                tT4[:, :, it, 0],
                xsum[:],
                mybir.ActivationFunctionType.Copy,
                scale=1.0 / HW,
            )
        return tT

    def vproj(g, tT):
        g0 = g * NG
        for jb in range(JC):
            psum = pv.tile([128, NG], F32, name="pvt", tag="pvt")
            for kc in range(KC):
                nc.tensor.matmul(
                    psum[:],
                    wv_sb[kc][:, jb * 128 : (jb + 1) * 128],
                    tT[:, kc * NG : (kc + 1) * NG],
                    start=(kc == 0),
                    stop=(kc == KC - 1),
                )
            # add vposT (same 50-col pattern for every item) + fp16 round
            nc.vector.tensor_add(
                vT[jb][:, g0 : g0 + NG].rearrange("p (i n) -> p i n", i=GI),
                psum[:].rearrange("p (i n) -> p i n", i=GI),
                vpos3[:, jb : jb + 1, :].broadcast_to((128, GI, N)),
            )

    def attention(g):
        g0 = g * NG
        # P[jb][c, it*50+m] = vT[c, it*50+m] * vT[c, it*50+0]
        psum_S = pS.tile([heads, NG], F32, name="psS", tag="psS")
        for jb in range(JC):
            vg3 = vT[jb][:, g0 : g0 + NG].rearrange("p (i n) -> p i n", i=GI)
            p = apool.tile([128, NG], F16, name="pprod", tag="pprod")
            nc.vector.tensor_mul(
                p[:].rearrange("p (i n) -> p i n", i=GI),
                vg3,
                vg3[:, :, 0:1].broadcast_to((128, GI, N)),
            )
            nc.tensor.matmul(
                psum_S[:],
                maskT_sb[:, jb * heads : (jb + 1) * heads],
                p[:],
                start=(jb == 0),
                stop=(jb == JC - 1),
            )
        # E = exp(S * scale), denominators per item block, A = E/D
        e_sb = apool.tile([heads, NG], F32, name="esb", tag="esb")
        nc.scalar.activation(
            e_sb[:], psum_S[:], mybir.ActivationFunctionType.Exp, scale=scale
        )
        d_sb = apool.tile([heads, GI], F32, name="dsb", tag="dsb")
        nc.vector.reduce_sum(
            d_sb[:],
            e_sb[:].rearrange("p (i n) -> p i n", i=GI),
            axis=mybir.AxisListType.X,
        )
        r_sb = apool.tile([heads, GI], F32, name="rsb", tag="rsb")
        nc.vector.reciprocal(r_sb[:], d_sb[:])
        a_sb = apool.tile([heads, NG], F16, name="asb", tag="asb")
        nc.vector.tensor_mul(
            a_sb[:].rearrange("p (i n) -> p i n", i=GI),
            e_sb[:].rearrange("p (i n) -> p i n", i=GI),
            r_sb[:].rearrange("p (i o) -> p i o", o=1).broadcast_to((heads, GI, N)),
        )
        # ctx[c] = sum_m A[head(c), m] vT[c, m]; write into token-0 cols
        for jb in range(JC):
            psum_a = pA.tile([128, NG], F32, name="psA", tag="psA")
            nc.tensor.matmul(
                psum_a[:],
                mask2_sb[:, jb * 128 : (jb + 1) * 128],
                a_sb[:],
                start=True,
                stop=True,
            )
            p2 = apool.tile([128, NG], F32, name="p2", tag="p2")
            nc.vector.tensor_mul(p2[:], psum_a[:], vT[jb][:, g0 : g0 + NG])
            ctx8 = apool.tile([128, GI], F32, name="ctx8", tag="ctx8")
            nc.vector.reduce_sum(
                ctx8[:],
                p2[:].rearrange("p (i n) -> p i n", i=GI),
                axis=mybir.AxisListType.X,
            )
            nc.scalar.activation(
                vT[jb][:, g0 : g0 + NG].rearrange("p (i n) -> p i n", i=GI)[
                    :, :, 0
                ],
                ctx8[:],
                mybir.ActivationFunctionType.Copy,
            )

    def outproj(mtiles):
        # out[tok, :] = uT.T @ WcT; token-stationary, 128 tokens per tile
        for m0, mw in mtiles:
            osb = opool.tile([128, OUT], F32, name="osb", tag="osb")
            for oc in range(OC2):
                psum = po.tile([128, 512], F32, name="pso", tag="pso")
                for kc in range(KC):
                    nc.tensor.matmul(
                        psum[:mw, :],
                        vT[kc][:, m0 : m0 + mw],
                        wc_sb[kc][:, oc * 512 : (oc + 1) * 512],
                        start=(kc == 0),
                        stop=(kc == KC - 1),
                    )
                nc.vector.tensor_copy(
                    osb[:mw, oc * 512 : (oc + 1) * 512], psum[:mw, :]
                )
            nc.sync.dma_start(out_flat[m0 : m0 + mw, :], osb[:mw, :])

    # Software-pipelined schedule: attention(g) PE work hides under
    # vproj(g+1); out-projection for tokens of groups 0..2 starts before
    # the last group attention completes.
    mtiles = [(m, min(128, TOK - m)) for m in range(0, TOK, 128)]
    early = [mt for mt in mtiles if mt[0] + mt[1] <= 3 * NG]
    late = [mt for mt in mtiles if mt[0] + mt[1] > 3 * NG]

    if variant == "full":
        tT0 = build_tT(0)
        vproj(0, tT0)
        tT1 = build_tT(1)
        vproj(1, tT1)
        attention(0)
        tT2 = build_tT(2)
        vproj(2, tT2)
        attention(1)
        tT3 = build_tT(3)
        vproj(3, tT3)
        attention(2)
        outproj(early)
        attention(3)
        outproj(late)
    elif variant == "vproj":
        for g in range(G):
            vproj(g, build_tT(g))
    elif variant == "vproj+attn":
        tT0 = build_tT(0)
        vproj(0, tT0)
        tT1 = build_tT(1)
        vproj(1, tT1)
        attention(0)
        tT2 = build_tT(2)
        vproj(2, tT2)
        attention(1)
        tT3 = build_tT(3)
        vproj(3, tT3)
        attention(2)
        attention(3)
    elif variant == "outproj":
        outproj(early)
        outproj(late)
    elif variant == "tT":
        for g in range(G):
            build_tT(g)


_NC_CACHE = {}
_RUN_CACHE = {}


def _get_nc(heads):
    if heads not in _NC_CACHE:
        _NC_CACHE[heads] = build_kernel(heads=heads)
    return _NC_CACHE[heads]


def _run(nc, in_maps):
    """run_bass_kernel_spmd equivalent (axon/PJRT path) with: the jitted
    executable cached across calls, weight-like inputs passed replicated
    (uploaded once, not 8x), and donated output buffers created on device
    (no zero upload)."""
    import jax
    import jax.numpy as jnp
    import numpy as _np
    from jax.sharding import Mesh, PartitionSpec, NamedSharding
    from jax.experimental.shard_map import shard_map
    import concourse.mybir as mb
    from concourse import bass2jax as b2j

    # inputs where every core got the identical array object -> replicated
    replicated = {
        nm
        for nm in in_maps[0]
        if all(m[nm] is in_maps[0][nm] for m in in_maps)
    }

    key = id(nc)
    if key not in _RUN_CACHE:
        b2j.install_neuronx_cc_hook()
        in_names, out_names, out_avals = [], [], []
        partition_name = (
            nc.partition_id_tensor.name if nc.partition_id_tensor else None
        )
        for alloc in nc.m.functions[0].allocations:
            if not isinstance(alloc, mb.MemoryLocationSet):
                continue
            name = alloc.memorylocations[0].name
            if alloc.kind == "ExternalInput":
                if name != partition_name:
                    in_names.append(name)
            elif alloc.kind == "ExternalOutput":
                shape = tuple(alloc.tensor_shape)
                dtype = mb.dt.np(alloc.dtype)
                out_names.append(name)
                out_avals.append(jax.core.ShapedArray(shape, dtype))
        n_params = len(in_names)
        n_outs = len(out_avals)
        all_names = list(in_names) + list(out_names)
        if partition_name is not None:
            all_names.append(partition_name)
        donate = tuple(range(n_params, n_params + n_outs))

        def _body(*args):
            operands = list(args)
            if partition_name is not None:
                operands.append(b2j.partition_id_tensor())
            outs = b2j._bass_exec_p.bind(
                *operands,
                out_avals=tuple(out_avals),
                in_names=tuple(all_names),
                out_names=tuple(out_names),
                lowering_input_output_aliases=(),
                sim_require_finite=True,
                sim_require_nnan=True,
                nc=nc,
            )
            return tuple(outs)

        devices = jax.devices()[:CORES]
        mesh = Mesh(_np.asarray(devices), ("core",))
        in_specs = tuple(
            PartitionSpec() if nm in replicated else PartitionSpec("core")
            for nm in in_names
        ) + (PartitionSpec("core"),) * n_outs
        out_specs = (PartitionSpec("core"),) * n_outs
        sharded = jax.jit(
            shard_map(
                _body, mesh=mesh, in_specs=in_specs, out_specs=out_specs,
                check_rep=False,
            ),
            donate_argnums=donate,
            keep_unused=True,
        )
        zeros_fns = [
            jax.jit(
                (lambda shape, dtype: lambda: jnp.zeros(shape, dtype))(
                    (CORES * av.shape[0], *av.shape[1:]), av.dtype
                ),
                out_shardings=NamedSharding(mesh, PartitionSpec("core")),
            )
            for av in out_avals
        ]
        _RUN_CACHE[key] = (
            sharded, in_names, out_names, out_avals, zeros_fns, replicated
        )

    sharded, in_names, out_names, out_avals, zeros_fns, replicated_c = (
        _RUN_CACHE[key]
    )
    assert replicated == replicated_c, "replication pattern changed"
    args = [
        _np.asarray(in_maps[0][nm])
        if nm in replicated
        else _np.concatenate([_np.asarray(m[nm]) for m in in_maps], axis=0)
        for nm in in_names
    ]
    dev_zeros = [f() for f in zeros_fns]
    out_arrs = sharded(*args, *dev_zeros)
    return [
        {
            nm: _np.asarray(out_arrs[i]).reshape(CORES, *out_avals[i].shape)[c]
            for i, nm in enumerate(out_names)
        }
        for c in range(CORES)
    ]


# ---------------------------------------------------------------- host side
def make_in_maps(inputs, heads=H):

    x = np.asarray(inputs["x"], np.float32)
    pos_emb = np.asarray(inputs["pos_emb"], np.float32)
    Wv = np.asarray(inputs["Wv"], np.float32)
    bv = np.asarray(inputs["bv"], np.float32)
    Wc = np.asarray(inputs["Wc"], np.float32)
    bc = np.asarray(inputs["bc"], np.float32)
    num_heads = int(np.asarray(inputs["num_heads"]))
    assert num_heads == heads and x.shape == (B, C, S, S)
    assert 1 <= heads <= 128 and C % heads == 0

    wvT = np.ascontiguousarray(Wv.T).astype(np.float16)
    wcT = np.ascontiguousarray(Wc.T).astype(np.float16)

    # vposT[128, kc*50 + n] = (pos_emb @ Wv.T + bv).T chunk-tiled
    vpos = (pos_emb @ Wv.T + bv).astype(np.float32)  # [N, C]
    vposT = np.empty((128, KC * N), np.float32)
    for kc in range(KC):
        vposT[:, kc * N : (kc + 1) * N] = vpos[:, kc * 128 : (kc + 1) * 128].T

    # maskT[p, kc*heads + h] = 1 if channel kc*128+p belongs to head h
    head_of = np.arange(C) // (C // heads)
    maskT = np.zeros((128, KC * heads), np.float16)
    mask2 = np.zeros((heads, KC * 128), np.float16)
    for kc in range(KC):
        for p in range(128):
            h = head_of[kc * 128 + p]
            maskT[p, kc * heads + h] = 1.0
            mask2[h, kc * 128 + p] = 1.0

    xr16 = np.ascontiguousarray(x.reshape(B, C, HW).astype(np.float16))
    in_maps = []
    for core in range(CORES):
        in_maps.append(
            {
                "x": xr16[core * IPC : (core + 1) * IPC],
                "wvT": wvT,
                "wcT": wcT,
                "vposT": vposT,
                "maskT": maskT,
                "mask2": mask2,
            }
        )

    return in_maps


def kernel(**inputs):
    from concourse._compat import axon_active

    heads = int(np.asarray(inputs["num_heads"]))
    in_maps = make_in_maps(inputs, heads)
    nc = _get_nc(heads)
    if axon_active():
        results = _run(nc, in_maps)
    else:
        results = run_bass_kernel_spmd(nc, in_maps, list(range(CORES))).results
    out = np.concatenate([results[i]["out"] for i in range(CORES)], axis=0)
    out = np.ascontiguousarray(out, dtype=np.float32)
    bc = np.asarray(inputs["bc"], np.float32)
    if bc.any():
        out = out + bc[None, None, :]
    return out
